# revision 1
# baseline (speedup 1.0000x reference)
"""BaiChuan attention block on 8 Trainium2 NeuronCores.

Sharding: tensor-parallel over heads (4 heads/core) for QKV projection and
attention; AllGather of attention outputs (feature-major) per batch; o_proj
column-sharded (each core computes a 512-wide output-feature slice for all
tokens); host concatenates slices.

Precision: Q/K path in bf16 (softmax output is insensitive to Q/K rounding
since probabilities only depend on score *differences*, which are tiny here);
V path, attention values, and o_proj in float32r (full-rate fp32 matmul mode,
~2e-4 relative error).
"""
import numpy as np
import ml_dtypes

import concourse.bass as bass
import concourse.mybir as mybir
import concourse.tile as tile
from concourse import bacc, bass_utils

# Problem dims (hardcoded per contest contract)
B, S, H, NH = 2, 2048, 4096, 32
D = H // NH            # 128 head dim
CORES = 8
HPC = NH // CORES      # 4 heads per core
TOK = B * S            # 4096 tokens
FQ = HPC * D           # 512 per-core q/k/v feature width
TCW = 512              # token chunk width for QKV phase
NTC = TOK // TCW       # 8 chunks
HC = H // 128          # 32 contraction chunks
QB = 512               # attention q block
ROPE_THETA = 10000.0

F32 = mybir.dt.float32
F32R = mybir.dt.float32r
BF16 = mybir.dt.bfloat16

_CACHE = {}
LAST_RESULTS = None


def _build():
    nc = bacc.Bacc("TRN2", target_bir_lowering=False, debug=False, num_devices=CORES)

    x = nc.dram_tensor("x", [TOK, H], F32R, kind="ExternalInput").ap()
    # wq/wk pre-split per 128-wide f-tile on host: [4, H, 128]
    wq = nc.dram_tensor("wq", [4, H, 128], BF16, kind="ExternalInput").ap()
    wk = nc.dram_tensor("wk", [4, H, 128], BF16, kind="ExternalInput").ap()
    wv = nc.dram_tensor("wv", [H, FQ], F32R, kind="ExternalInput").ap()
    wo = nc.dram_tensor("wo", [H, FQ], BF16, kind="ExternalInput").ap()
    cosq = nc.dram_tensor("cosq", [128, TOK], F32, kind="ExternalInput").ap()
    sinq = nc.dram_tensor("sinq", [128, TOK], F32, kind="ExternalInput").ap()
    cosk = nc.dram_tensor("cosk", [128, TOK], F32, kind="ExternalInput").ap()
    sink = nc.dram_tensor("sink", [128, TOK], F32, kind="ExternalInput").ap()
    masks = nc.dram_tensor("masks", [128, 4, QB], F32, kind="ExternalInput").ap()
    ones_col = nc.dram_tensor("ones_col", [128, 1], F32R, kind="ExternalInput").ap()
    ones_row = nc.dram_tensor("ones_row", [1, 128], F32R, kind="ExternalInput").ap()
    ident = nc.dram_tensor("ident", [128, 128], F32R, kind="ExternalInput").ap()
    out = nc.dram_tensor("out", [TOK, FQ], F32, kind="ExternalOutput").ap()

    with tile.TileContext(nc) as tc, nc.allow_low_precision(reason="f32r/bf16 kernel"):
        with tc.tile_pool(name="dram", bufs=1, space="DRAM") as dram, \
             tc.tile_pool(name="const", bufs=1) as constp:
            qt = [dram.tile([FQ, S], BF16, name=f"qt{b_}") for b_ in range(B)]
            kt = [dram.tile([FQ, S], BF16, name=f"kt{b_}") for b_ in range(B)]
            vv = [dram.tile([S, FQ], F32R, name=f"vv{b_}") for b_ in range(B)]
            aloc = [dram.tile([FQ, S], BF16, name=f"aloc{b_}") for b_ in range(B)]
            agth = [dram.tile([H, S], BF16, name=f"agth{b_}") for b_ in range(B)]

            ones_sb = constp.tile([128, 1], F32R)
            ones_row_sb = constp.tile([1, 128], F32R)
            ident_sb = constp.tile([128, 128], F32R)
            mask_sb = constp.tile([128, 4, QB], F32)
            nc.sync.dma_start(ones_sb[:], ones_col)
            nc.sync.dma_start(ones_row_sb[:], ones_row)
            nc.sync.dma_start(ident_sb[:], ident)
            nc.sync.dma_start(mask_sb[:], masks)

            with tc.tile_pool(name="wqf", bufs=3) as wqfp, \
                 tc.tile_pool(name="wvs", bufs=3) as wvsp, \
                 tc.tile_pool(name="xtp", bufs=2) as xtp, \
                 tc.tile_pool(name="xin", bufs=2) as xinp, \
                 tc.tile_pool(name="xtr", bufs=2) as xtrp, \
                 tc.tile_pool(name="tab", bufs=1) as tabp, \
                 tc.tile_pool(name="qev", bufs=3) as evp, \
                 tc.tile_pool(name="akv", bufs=2) as akvp, \
                 tc.tile_pool(name="att", bufs=3) as attp, \
                 tc.tile_pool(name="ptr", bufs=2, space="PSUM") as ptrp, \
                 tc.tile_pool(name="pqk", bufs=2, space="PSUM") as pqkp, \
                 tc.tile_pool(name="pvp", bufs=1, space="PSUM") as pvp:

                def qkv_chunk(tci):
                    t0 = tci * TCW
                    b_c = t0 // S
                    tl = t0 % S
                    cq = tabp.tile([128, TCW], F32, tag="cq", name="cq")
                    sq_ = tabp.tile([128, TCW], F32, tag="sq", name="sq")
                    ck = tabp.tile([128, TCW], F32, tag="ck", name="ck")
                    sk_ = tabp.tile([128, TCW], F32, tag="sk", name="sk")
                    nc.sync.dma_start(cq[:], cosq[:, t0:t0 + TCW])
                    nc.sync.dma_start(sq_[:], sinq[:, t0:t0 + TCW])
                    nc.sync.dma_start(ck[:], cosk[:, t0:t0 + TCW])
                    nc.sync.dma_start(sk_[:], sink[:, t0:t0 + TCW])

                    xt_bf = xtp.tile([128, HC, TCW], BF16, tag="xtbf", name="xtbf")
                    pv_t = [pvp.tile([128, FQ], F32, tag=f"pv{ts}", name=f"pv{ts}")
                            for ts in range(4)]

                    for hc in range(HC):
                        xblk = xinp.tile([128, 4, 128], F32R, tag="xblk", name="xblk")
                        src = x[t0:t0 + TCW, hc * 128:(hc + 1) * 128]
                        nc.sync.dma_start(
                            xblk[:], src.rearrange("(a p) e -> p a e", p=128))
                        ptr_ = ptrp.tile([128, 4, 128], F32R, tag="ptr", name="ptr")
                        for ts in range(4):
                            nc.tensor.transpose(
                                ptr_[:, ts, :], xblk[:, ts, :], ident_sb[:])
                        nc.any.tensor_copy(
                            xt_bf[:, hc, :],
                            ptr_.rearrange("p a e -> p (a e)"))
                        xtr = xtrp.tile([128, 4, 128], F32R, tag="xtr", name="xtr")
                        nc.any.tensor_copy(xtr[:], ptr_[:])
                        wvs = wvsp.tile([128, FQ], F32R, tag="wvs", name="wvs")
                        nc.sync.dma_start(wvs[:], wv[hc * 128:(hc + 1) * 128, :])
                        for ts in range(4):
                            nc.tensor.matmul(
                                pv_t[ts][:], xtr[:, ts, :], wvs[:],
                                start=(hc == 0), stop=(hc == HC - 1))

                    for ts in range(4):
                        vout = evp.tile([128, FQ], F32R, tag="vout", name="vout")
                        nc.any.tensor_copy(vout[:], pv_t[ts][:])
                        nc.sync.dma_start(
                            vv[b_c][tl + ts * 128:tl + (ts + 1) * 128, :], vout[:])

                    for f in range(8):
                        w_dram = wq if f < 4 else wk
                        fi = f % 4
                        wqf = wqfp.tile([128, HC, 128], BF16, tag="wqf", name="wqf")
                        nc.sync.dma_start(
                            wqf[:], w_dram[fi].rearrange("(k p) e -> p k e", p=128))
                        pqk = pqkp.tile([128, TCW], F32, tag="pqk", name="pqk")
                        for hc in range(HC):
                            nc.tensor.matmul(
                                pqk[:], wqf[:, hc, :], xt_bf[:, hc, :],
                                start=(hc == 0), stop=(hc == HC - 1))
                        cos_t = cq if f < 4 else ck
                        sin_t = sq_ if f < 4 else sk_
                        tmp = evp.tile([128, TCW], F32, tag="tmp", name="tmp")
                        sw = evp.tile([128, TCW], F32, tag="sw", name="sw")
                        oev = evp.tile([128, TCW], BF16, tag="oev", name="oev")
                        nc.vector.tensor_mul(tmp[:], pqk[:], cos_t[:])
                        nc.vector.tensor_mul(sw[0:64, :], pqk[64:128, :], sin_t[0:64, :])
                        nc.vector.tensor_mul(sw[64:128, :], pqk[0:64, :], sin_t[64:128, :])
                        nc.vector.tensor_add(oev[:], tmp[:], sw[:])
                        dst = (qt if f < 4 else kt)[b_c]
                        nc.sync.dma_start(
                            dst[fi * 128:(fi + 1) * 128, tl:tl + TCW], oev[:])

                def attn_batch(b_i):
                    for hl in range(HPC):
                        r0 = hl * 128
                        kt_sb = akvp.tile([128, S], BF16, tag="kt", name="kt_sb")
                        nc.scalar.dma_start(kt_sb[:], kt[b_i][r0:r0 + 128, :])
                        v_sb = akvp.tile([128, S // 128, 128], F32R, tag="v", name="v_sb")
                        nc.scalar.dma_start(
                            v_sb[:],
                            vv[b_i][:, r0:r0 + 128].rearrange("(c p) e -> p c e", p=128))
                        for j in range(S // QB):
                            q_sb = attp.tile([128, QB], BF16, tag="q", name="q_sb")
                            nc.scalar.dma_start(
                                q_sb[:], qt[b_i][r0:r0 + 128, j * QB:(j + 1) * QB])
                            po = pvp.tile([128, QB], F32, tag="pv0", name="po")
                            ps = pvp.tile([1, QB], F32, tag="pv1", name="ps")
                            nkv = 4 * (j + 1)
                            for c in range(nkv):
                                pst_tag = ("pv2", "pv3", "pqk")[c % 3]
                                pst = (pqkp if c % 3 == 2 else pvp).tile(
                                    [128, QB], F32, tag=pst_tag, name="pst")
                                nc.tensor.matmul(
                                    pst[:], kt_sb[:, c * 128:(c + 1) * 128],
                                    q_sb[:], start=True, stop=True)
                                dr = c - 4 * j
                                pt = attp.tile([128, QB], F32R, tag="pt", name="pt")
                                if dr >= 0:
                                    et = attp.tile([128, QB], F32, tag="et", name="et")
                                    nc.scalar.activation(
                                        et[:], pst[:],
                                        mybir.ActivationFunctionType.Exp)
                                    nc.vector.tensor_mul(
                                        pt[:], et[:], mask_sb[:, dr, :])
                                else:
                                    nc.scalar.activation(
                                        pt[:], pst[:],
                                        mybir.ActivationFunctionType.Exp)
                                nc.tensor.matmul(
                                    po[:], v_sb[:, c, :], pt[:],
                                    start=(c == 0), stop=(c == nkv - 1))
                                nc.tensor.matmul(
                                    ps[:], ones_sb[:], pt[:],
                                    start=(c == 0), stop=(c == nkv - 1))
                            r_sb = attp.tile([1, QB], F32R, tag="r", name="r_sb")
                            nc.vector.reciprocal(r_sb[:], ps[:])
                            pb = ptrp.tile([128, QB], F32, tag="ptr", name="pb")
                            nc.tensor.matmul(
                                pb[:], ones_row_sb[:], r_sb[:], start=True, stop=True)
                            bsb = attp.tile([128, QB], F32, tag="bsb", name="bsb")
                            nc.vector.tensor_copy(bsb[:], pb[:])
                            o_sb = attp.tile([128, QB], BF16, tag="osb", name="o_sb")
                            nc.vector.tensor_mul(o_sb[:], po[:], bsb[:])
                            nc.scalar.dma_start(
                                aloc[b_i][r0:r0 + 128, j * QB:(j + 1) * QB], o_sb[:])
                    nc.gpsimd.collective_compute(
                        "AllGather",
                        mybir.AluOpType.bypass,
                        ins=[aloc[b_i].opt()],
                        outs=[agth[b_i].opt()],
                        replica_groups=[list(range(CORES))],
                    )

                def oproj_batch(b_i):
                    for ttg in range(S // QB):
                        pfs = [pvp.tile([128, FQ], F32, tag=f"pv{tt}", name=f"pf{tt}")
                               for tt in range(4)]
                        for k in range(HC):
                            wo_t = wqfp.tile([128, FQ], BF16, tag="wqf", name="wo_t")
                            nc.sync.dma_start(
                                wo_t[:], wo[k * 128:(k + 1) * 128, :])
                            agr = evp.tile([128, QB], BF16, tag="oev", name="agr")
                            nc.sync.dma_start(
                                agr[:],
                                agth[b_i][k * 128:(k + 1) * 128,
                                          ttg * QB:(ttg + 1) * QB])
                            for tt in range(4):
                                nc.tensor.matmul(
                                    pfs[tt][:], agr[:, tt * 128:(tt + 1) * 128],
                                    wo_t[:], start=(k == 0), stop=(k == HC - 1))
                        for tt in range(4):
                            fo = evp.tile([128, FQ], F32, tag="tmp", name="fo")
                            nc.any.tensor_copy(fo[:], pfs[tt][:])
                            t_row = b_i * S + ttg * QB + tt * 128
                            nc.sync.dma_start(out[t_row:t_row + 128, :], fo[:])

                with nc.named_scope("qkv_a"):
                    for tci in range(4):
                        qkv_chunk(tci)
                with nc.named_scope("attn0"):
                    attn_batch(0)
                with nc.named_scope("qkv_b"):
                    for tci in range(4, 8):
                        qkv_chunk(tci)
                with nc.named_scope("attn1"):
                    attn_batch(1)
                with nc.named_scope("oproj0"):
                    oproj_batch(0)
                with nc.named_scope("oproj1"):
                    oproj_batch(1)

    nc.compile()
    return nc


def _get_nc():
    if "nc" not in _CACHE:
        _CACHE["nc"] = _build()
    return _CACHE["nc"]


def kernel(positions, hidden_states, w_pack, w_o):
    global LAST_RESULTS
    nc = _get_nc()

    x = np.ascontiguousarray(
        np.asarray(hidden_states, dtype=np.float32).reshape(TOK, H))
    w_pack = np.asarray(w_pack, dtype=np.float32)
    w_o = np.asarray(w_o, dtype=np.float32)
    pos_flat = np.asarray(positions).reshape(-1).astype(np.float64)  # [TOK]

    half = D // 2
    inv = 1.0 / (ROPE_THETA ** (np.arange(half, dtype=np.float64) * 2.0 / D))
    f = np.outer(inv, pos_flat)                        # [64, TOK]
    cos = np.cos(f)
    sin = np.sin(f)
    cos_t = np.concatenate([cos, cos], axis=0)         # [128, TOK]
    sin_t = np.concatenate([-sin, sin], axis=0)
    scale = D ** -0.5
    cosq = (cos_t * scale).astype(np.float32)
    sinq = (sin_t * scale).astype(np.float32)
    cosk = cos_t.astype(np.float32)
    sink = sin_t.astype(np.float32)

    kvi = np.arange(128)[:, None, None]
    rr = np.arange(4)[None, :, None]
    qi = np.arange(QB)[None, None, :]
    masks = ((kvi + 128 * rr) <= qi).astype(np.float32)

    ones_col = np.ones((128, 1), np.float32)
    ones_row = np.ones((1, 128), np.float32)
    ident = np.eye(128, dtype=np.float32)

    in_maps = []
    for c in range(CORES):
        in_maps.append({
            "x": x,
            "wq": np.ascontiguousarray(
                w_pack[:, FQ * c:FQ * (c + 1)].reshape(H, 4, 128).transpose(1, 0, 2)
            ).astype(ml_dtypes.bfloat16),
            "wk": np.ascontiguousarray(
                w_pack[:, H + FQ * c:H + FQ * (c + 1)].reshape(H, 4, 128)
                .transpose(1, 0, 2)).astype(ml_dtypes.bfloat16),
            "wv": np.ascontiguousarray(w_pack[:, 2 * H + FQ * c:2 * H + FQ * (c + 1)]),
            "wo": np.ascontiguousarray(w_o[:, FQ * c:FQ * (c + 1)]).astype(ml_dtypes.bfloat16),
            "cosq": cosq, "sinq": sinq, "cosk": cosk, "sink": sink,
            "masks": masks, "ones_col": ones_col, "ones_row": ones_row,
            "ident": ident,
        })

    res = bass_utils.run_bass_kernel_spmd(nc, in_maps, core_ids=list(range(CORES)))
    LAST_RESULTS = res
    outs = [res.results[c]["out"] for c in range(CORES)]
    return np.concatenate(outs, axis=1).reshape(B, S, H)



# revision 7
# speedup vs baseline: 1.4524x; 1.4524x over previous
"""BaiChuan attention block on 8 Trainium2 NeuronCores.

Sharding: tensor-parallel over heads (4 heads/core). Each core computes its
512-wide q/k/v slices for all 4096 tokens, runs attention for its 4 heads on
both batches, AllGathers attention outputs (feature-major, bf16) per
half-batch, and computes a 512-wide output-feature slice of o_proj for all
tokens; the host concatenates slices.

Layout strategy: x is pre-transposed on the host to feature-major (xT), so no
PE transposes are needed anywhere. Weights live in SBUF for the whole kernel
(loaded once). q/k/v stay in SBUF per batch (no DRAM roundtrip).

Precision: Q/K projection runs in fp8 (e4m3) with DoubleRow packing; the
softmax only depends on score differences, which are tiny for this data, so
fp8 rounding of q/k is far below the output tolerance. Scales: x*XS and w*WS
are folded out through the RoPE tables; q/k are stored in SBUF as fp8 scaled
by QS, and D**-0.5 / QS**2 is applied via the exp's scale argument. The V
path, attention values, and o_proj run in bf16 with fp32 PSUM accumulation.
"""
import numpy as np
import ml_dtypes

import concourse.bass as bass
import concourse.mybir as mybir
import concourse.tile as tile
from concourse import bacc, bass_utils

# Problem dims (hardcoded per contest contract)
B, S, H, NH = 2, 2048, 4096, 32
D = H // NH            # 128 head dim
CORES = 8
HPC = NH // CORES      # 4 heads per core
TOK = B * S            # 4096 tokens
FQ = HPC * D           # 512 per-core q/k/v feature width
TCW = 512              # token chunk width for QKV phase
NTC = S // TCW         # 4 chunks per batch
HC = H // 128          # 32 contraction chunks
QB = 512               # attention q block
ROPE_THETA = 10000.0

# fp8 scale plan
XS = 32.0              # x pre-scale before fp8 quantization
WS = 32.0              # wq/wk pre-scale before fp8 quantization
QS = 16.0              # q/k SBUF storage scale
SEXP = float(D ** -0.5 / (QS * QS))  # exp() input scale

F32 = mybir.dt.float32
F32R = mybir.dt.float32r
BF16 = mybir.dt.bfloat16
F8 = mybir.dt.float8e4
DR = mybir.MatmulPerfMode.DoubleRow

_CACHE = {}
LAST_RESULTS = None


def _build():
    nc = bacc.Bacc("TRN2", target_bir_lowering=False, debug=False, num_devices=CORES)

    # [128, NCHUNK, HC, TCW]: partition = feature-within-chunk, pre-chunked on
    # host so every per-chunk DMA is contiguous per partition
    xq8 = nc.dram_tensor("xq8", [128, B * NTC, HC, TCW], F8, kind="ExternalInput").ap()
    xbf = nc.dram_tensor("xbf", [128, B * NTC, HC, TCW], BF16, kind="ExternalInput").ap()
    # weights [128, HC, FQ]: partition = contraction-feature-within-chunk
    wq8 = nc.dram_tensor("wq8", [128, HC, FQ], F8, kind="ExternalInput").ap()
    wk8 = nc.dram_tensor("wk8", [128, HC, FQ], F8, kind="ExternalInput").ap()
    wv = nc.dram_tensor("wv", [128, HC, FQ], BF16, kind="ExternalInput").ap()
    wo = nc.dram_tensor("wo", [128, HC, FQ], BF16, kind="ExternalInput").ap()
    cosT = nc.dram_tensor("cosT", [128, TOK], BF16, kind="ExternalInput").ap()
    sinT = nc.dram_tensor("sinT", [128, TOK], BF16, kind="ExternalInput").ap()
    masks = nc.dram_tensor("masks", [128, 4, QB], BF16, kind="ExternalInput").ap()
    ones_col = nc.dram_tensor("ones_col", [128, 1], BF16, kind="ExternalInput").ap()
    ones_row = nc.dram_tensor("ones_row", [1, 128], F32R, kind="ExternalInput").ap()
    out = nc.dram_tensor("out", [TOK, FQ], F32, kind="ExternalOutput").ap()

    with tile.TileContext(nc) as tc, nc.allow_low_precision(reason="fp8/bf16 kernel"):
        with tc.tile_pool(name="dram", bufs=1, space="DRAM") as dram, \
             tc.tile_pool(name="dsh", bufs=1, space="DRAM") as dsh, \
             tc.tile_pool(name="wconst", bufs=1) as wconst:
            # aloc[b][h2]: attention out for one half-batch (feature-major)
            aloc = [[dram.tile([FQ, 2 * QB], BF16, name=f"aloc{b_}{h_}")
                     for h_ in range(2)] for b_ in range(B)]
            agth = [[dsh.tile([H, 2 * QB], BF16, addr_space="Shared",
                              name=f"agth{b_}{h_}")
                     for h_ in range(2)] for b_ in range(B)]

            # resident weights + small constants
            wq_sb = wconst.tile([128, HC, FQ], F8)
            wk_sb = wconst.tile([128, HC, FQ], F8)
            wv_sb = wconst.tile([128, HC, FQ], BF16)
            wo_sb = wconst.tile([128, HC, FQ], BF16)
            mask_sb = wconst.tile([128, 4, QB], BF16)
            ones_sb = wconst.tile([128, 1], BF16)
            onesr_sb = wconst.tile([1, 128], F32R)
            nc.sync.dma_start(wq_sb[:], wq8)
            nc.sync.dma_start(wk_sb[:], wk8)
            nc.sync.dma_start(wv_sb[:], wv)
            nc.sync.dma_start(wo_sb[:], wo)
            nc.sync.dma_start(mask_sb[:], masks)
            nc.sync.dma_start(ones_sb[:], ones_col)
            nc.sync.dma_start(onesr_sb[:], ones_row)

            with tc.tile_pool(name="xq", bufs=2) as xqp, \
                 tc.tile_pool(name="xb", bufs=4) as xbp, \
                 tc.tile_pool(name="tbl", bufs=2) as tblp, \
                 tc.tile_pool(name="qkv", bufs=1) as qkvp, \
                 tc.tile_pool(name="ev", bufs=2) as evp, \
                 tc.tile_pool(name="att", bufs=2) as attp, \
                 tc.tile_pool(name="psA", bufs=2, space="PSUM") as psA, \
                 tc.tile_pool(name="psB", bufs=2, space="PSUM") as psB, \
                 tc.tile_pool(name="psS", bufs=1, space="PSUM") as psS:

                # per-batch SBUF q/k/v (reused between batches, bufs=1)
                q_sb = qkvp.tile([128, HPC, S], F8, name="q_sb")
                k_sb = qkvp.tile([128, HPC, S], F8, name="k_sb")
                v_sb = qkvp.tile([128, S // 128, FQ], BF16, name="v_sb")

                def qkv_batch(b_i):
                    for ch in range(NTC):
                        t0 = b_i * S + ch * TCW
                        cidx = b_i * NTC + ch
                        xq = xqp.tile([128, HC, TCW], F8, tag="xq", name="xq")
                        nc.sync.dma_start(xq[:], xq8[:, cidx, :, :])
                        cs = tblp.tile([128, TCW], BF16, tag="cs", name="cs")
                        sn = tblp.tile([128, TCW], BF16, tag="sn", name="sn")
                        nc.sync.dma_start(cs[:], cosT[:, t0:t0 + TCW])
                        nc.sync.dma_start(sn[:], sinT[:, t0:t0 + TCW])

                        # Q/K projection: fp8 DoubleRow, out [f=128, tok=512]
                        for f in range(8):
                            w_sb = wq_sb if f < 4 else wk_sb
                            hl = f % 4
                            col = hl * 128
                            pqk = psA.tile([128, TCW], F32, tag="qk", name="pqk")
                            for h2 in range(HC // 2):
                                nc.tensor.matmul(
                                    pqk[:],
                                    w_sb[:, 2 * h2:2 * h2 + 2, col:col + 128],
                                    xq[:, 2 * h2:2 * h2 + 2, :],
                                    start=(h2 == 0), stop=(h2 == HC // 2 - 1),
                                    perf_mode=DR)
                            # RoPE (neox rotate-half via sign-folded sin table)
                            tmp = evp.tile([128, TCW], F32, tag="tmp", name="tmp")
                            sw = evp.tile([128, TCW], F32, tag="sw", name="sw")
                            nc.vector.tensor_mul(tmp[:], pqk[:], cs[:])
                            nc.vector.tensor_mul(sw[0:64, :], pqk[64:128, :], sn[0:64, :])
                            nc.vector.tensor_mul(sw[64:128, :], pqk[0:64, :], sn[64:128, :])
                            dst = (q_sb if f < 4 else k_sb)
                            nc.vector.tensor_add(
                                dst[:, hl, ch * TCW:(ch + 1) * TCW], tmp[:], sw[:])

                        # V projection: bf16, out [tok=128, f=512] per tok tile
                        pv = [psB.tile([128, 2, FQ], F32, tag="sc", name=f"pv{i}")
                              for i in range(2)]
                        for hc in range(HC):
                            xb = xbp.tile([128, TCW], BF16, tag="xb", name="xb")
                            nc.sync.dma_start(xb[:], xbf[:, cidx, hc, :])
                            for ts in range(4):
                                nc.tensor.matmul(
                                    pv[ts // 2][:, ts % 2, :],
                                    xb[:, ts * 128:(ts + 1) * 128],
                                    wv_sb[:, hc, :],
                                    start=(hc == 0), stop=(hc == HC - 1))
                        for ts in range(4):
                            nc.scalar.copy(
                                v_sb[:, ch * 4 + ts, :], pv[ts // 2][:, ts % 2, :])

                def attn_batch(b_i):
                    for j in range(S // QB):
                        for hl in range(HPC):
                            q_rhs = q_sb[:, hl, j * QB:(j + 1) * QB]
                            po = psA.tile([128, QB], F32, tag="qk", name="po")
                            ps = psS.tile([1, QB], F32, tag="ps", name="ps")
                            npair = 2 * (j + 1)
                            for p in range(npair):
                                sc = psB.tile([128, 2, QB], F32, tag="sc", name="sc")
                                for ci in range(2):
                                    c = 2 * p + ci
                                    nc.tensor.matmul(
                                        sc[:, ci, :],
                                        k_sb[:, hl, c * 128:(c + 1) * 128],
                                        q_rhs, start=True, stop=True)
                                pt = attp.tile([128, 2, QB], BF16, tag="pt", name="pt")
                                dr0 = 2 * p - 4 * j
                                if dr0 >= 0:
                                    et = attp.tile([128, 2, QB], BF16, tag="et", name="et")
                                    nc.scalar.activation(
                                        et[:], sc[:],
                                        mybir.ActivationFunctionType.Exp,
                                        scale=SEXP)
                                    nc.vector.tensor_mul(
                                        pt[:], et[:], mask_sb[:, dr0:dr0 + 2, :])
                                else:
                                    nc.scalar.activation(
                                        pt[:], sc[:],
                                        mybir.ActivationFunctionType.Exp,
                                        scale=SEXP)
                                for ci in range(2):
                                    c = 2 * p + ci
                                    first = (p == 0 and ci == 0)
                                    last = (p == npair - 1 and ci == 1)
                                    nc.tensor.matmul(
                                        po[:], v_sb[:, c, hl * 128:(hl + 1) * 128],
                                        pt[:, ci, :], start=first, stop=last)
                                    nc.tensor.matmul(
                                        ps[:], ones_sb[:], pt[:, ci, :],
                                        start=first, stop=last)
                            r_sb = attp.tile([1, QB], F32R, tag="r", name="r_sb")
                            nc.vector.reciprocal(r_sb[:], ps[:])
                            pb = psA.tile([128, QB], F32, tag="qk", name="pb")
                            nc.tensor.matmul(
                                pb[:], onesr_sb[:], r_sb[:], start=True, stop=True)
                            bsb = attp.tile([128, QB], F32, tag="bsb", name="bsb")
                            nc.vector.tensor_copy(bsb[:], pb[:])
                            o_sb = attp.tile([128, QB], BF16, tag="osb", name="o_sb")
                            nc.vector.tensor_mul(o_sb[:], po[:], bsb[:])
                            nc.sync.dma_start(
                                aloc[b_i][j // 2][hl * 128:(hl + 1) * 128,
                                                  (j % 2) * QB:(j % 2 + 1) * QB],
                                o_sb[:])
                        if j % 2 == 1:
                            nc.gpsimd.collective_compute(
                                "AllGather",
                                mybir.AluOpType.bypass,
                                ins=[aloc[b_i][j // 2].opt()],
                                outs=[agth[b_i][j // 2].opt()],
                                replica_groups=[list(range(CORES))],
                            )

                def oproj_batch(b_i):
                    for h2 in range(2):
                        for tt in range(2):
                            pf = [psB.tile([128, 2, FQ], F32, tag="sc", name=f"pf{i}")
                                  for i in range(2)]
                            for k in range(HC):
                                agr = evp.tile([128, QB], BF16, tag="agr", name="agr")
                                nc.sync.dma_start(
                                    agr[:],
                                    agth[b_i][h2][k * 128:(k + 1) * 128,
                                                  tt * QB:(tt + 1) * QB])
                                for ts in range(4):
                                    nc.tensor.matmul(
                                        pf[ts // 2][:, ts % 2, :],
                                        agr[:, ts * 128:(ts + 1) * 128],
                                        wo_sb[:, k, :],
                                        start=(k == 0), stop=(k == HC - 1))
                            t0 = b_i * S + h2 * 2 * QB + tt * QB
                            for ts in range(4):
                                fo = evp.tile([128, FQ], F32, tag="fo", name="fo")
                                nc.scalar.copy(fo[:], pf[ts // 2][:, ts % 2, :])
                                nc.sync.dma_start(
                                    out[t0 + ts * 128:t0 + (ts + 1) * 128, :], fo[:])

                with nc.named_scope("qkv_a"):
                    qkv_batch(0)
                with nc.named_scope("attn0"):
                    attn_batch(0)
                with nc.named_scope("qkv_b"):
                    qkv_batch(1)
                with nc.named_scope("attn1"):
                    attn_batch(1)
                with nc.named_scope("oproj0"):
                    oproj_batch(0)
                with nc.named_scope("oproj1"):
                    oproj_batch(1)

    nc.compile()
    return nc


def _get_nc():
    if "nc" not in _CACHE:
        _CACHE["nc"] = _build()
    return _CACHE["nc"]


def _chunked(a):
    """[H, N] -> [128, HC, N] with dim1 = feature chunk."""
    return np.ascontiguousarray(
        a.reshape(HC, 128, a.shape[1]).transpose(1, 0, 2))


def _chunked_x(a):
    """[H, TOK] -> [128, B*NTC, HC, TCW] (token-chunked, feature-chunked)."""
    return np.ascontiguousarray(
        a.reshape(HC, 128, B * NTC, TCW).transpose(1, 2, 0, 3))


def kernel(positions, hidden_states, w_pack, w_o):
    global LAST_RESULTS
    nc = _get_nc()

    x = np.asarray(hidden_states, dtype=np.float32).reshape(TOK, H)
    w_pack = np.asarray(w_pack, dtype=np.float32)
    w_o = np.asarray(w_o, dtype=np.float32)
    pos_flat = np.asarray(positions).reshape(-1).astype(np.float64)  # [TOK]

    xT = x.T  # [H, TOK]
    xq8_full = _chunked_x((xT * XS).astype(ml_dtypes.float8_e4m3))
    xbf_full = _chunked_x(xT.astype(ml_dtypes.bfloat16))

    half = D // 2
    inv = 1.0 / (ROPE_THETA ** (np.arange(half, dtype=np.float64) * 2.0 / D))
    f = np.outer(inv, pos_flat)                        # [64, TOK]
    cos = np.cos(f)
    sin = np.sin(f)
    tscale = QS / (XS * WS)
    cosT = (np.concatenate([cos, cos], axis=0) * tscale).astype(ml_dtypes.bfloat16)
    sinT = (np.concatenate([-sin, sin], axis=0) * tscale).astype(ml_dtypes.bfloat16)

    kvi = np.arange(128)[:, None, None]
    rr = np.arange(4)[None, :, None]
    qi = np.arange(QB)[None, None, :]
    masks = ((kvi + 128 * rr) <= qi).astype(ml_dtypes.bfloat16)

    ones_col = np.ones((128, 1), ml_dtypes.bfloat16)
    ones_row = np.ones((1, 128), np.float32)

    in_maps = []
    for c in range(CORES):
        wq = (w_pack[:, FQ * c:FQ * (c + 1)] * WS).astype(ml_dtypes.float8_e4m3)
        wk = (w_pack[:, H + FQ * c:H + FQ * (c + 1)] * WS).astype(ml_dtypes.float8_e4m3)
        wvc = w_pack[:, 2 * H + FQ * c:2 * H + FQ * (c + 1)].astype(ml_dtypes.bfloat16)
        woc = w_o[:, FQ * c:FQ * (c + 1)].astype(ml_dtypes.bfloat16)
        in_maps.append({
            "xq8": xq8_full,
            "xbf": xbf_full,
            "wq8": _chunked(wq),
            "wk8": _chunked(wk),
            "wv": _chunked(wvc),
            "wo": _chunked(woc),
            "cosT": cosT, "sinT": sinT,
            "masks": masks, "ones_col": ones_col, "ones_row": ones_row,
        })

    res = bass_utils.run_bass_kernel_spmd(nc, in_maps, core_ids=list(range(CORES)))
    LAST_RESULTS = res
    outs = [res.results[c]["out"] for c in range(CORES)]
    return np.concatenate(outs, axis=1).reshape(B, S, H)


# revision 11
# speedup vs baseline: 1.5810x; 1.0885x over previous
"""BaiChuan attention block on 8 Trainium2 NeuronCores.

Sharding: tensor-parallel over heads (4 heads/core). Each core computes its
512-wide q/k/v slices for all 4096 tokens, runs attention for its 4 heads on
both batches, AllGathers attention outputs (feature-major, bf16) per
half-batch, and computes a 512-wide output-feature slice of o_proj for all
tokens; the host concatenates slices.

Layout strategy: x is pre-transposed on the host to feature-major (xT), so no
PE transposes are needed anywhere. Weights live in SBUF for the whole kernel
(loaded once). q/k/v stay in SBUF per batch (no DRAM roundtrip).

Precision: Q/K projection runs in fp8 (e4m3) with DoubleRow packing; the
softmax only depends on score differences, which are tiny for this data, so
fp8 rounding of q/k is far below the output tolerance. Scales: x*XS and w*WS
are folded out through the RoPE tables; q/k are stored in SBUF as fp8 scaled
by QS, and D**-0.5 / QS**2 is applied via the exp's scale argument. The V
path, attention values, and o_proj run in bf16 with fp32 PSUM accumulation.

Attention is software-pipelined: score matmuls run one kv-pair ahead of the
exp/PV/sum consumers, and each block's normalization (reciprocal + broadcast)
is deferred one pair-slot so the PE never waits on the DVE chain.
"""
import numpy as np
import ml_dtypes

import concourse.bass as bass
import concourse.mybir as mybir
import concourse.tile as tile
from concourse import bacc, bass_utils

# Problem dims (hardcoded per contest contract)
B, S, H, NH = 2, 2048, 4096, 32
D = H // NH            # 128 head dim
CORES = 8
HPC = NH // CORES      # 4 heads per core
TOK = B * S            # 4096 tokens
FQ = HPC * D           # 512 per-core q/k/v feature width
TCW = 512              # token chunk width for QKV phase
NTC = S // TCW         # 4 chunks per batch
HC = H // 128          # 32 contraction chunks
QB = 512               # attention q block
ROPE_THETA = 10000.0

# fp8 scale plan
XS = 32.0              # x pre-scale before fp8 quantization
WS = 32.0              # wq/wk pre-scale before fp8 quantization
QS = 16.0              # q/k SBUF storage scale
SEXP = float(D ** -0.5 / (QS * QS))  # exp() input scale

F32 = mybir.dt.float32
F32R = mybir.dt.float32r
BF16 = mybir.dt.bfloat16
F8 = mybir.dt.float8e4
DR = mybir.MatmulPerfMode.DoubleRow

_CACHE = {}
LAST_RESULTS = None


def _build():
    nc = bacc.Bacc("TRN2", target_bir_lowering=False, debug=False, num_devices=CORES)

    # x: [128, NCHUNK, HC, TCW], pre-chunked so per-chunk DMAs are contiguous
    xq8 = nc.dram_tensor("xq8", [128, B * NTC, HC, TCW], F8, kind="ExternalInput").ap()
    xbf = nc.dram_tensor("xbf", [128, B * NTC, HC, TCW], BF16, kind="ExternalInput").ap()
    # wq/wk: head-major [4, 128, HC, 128] so per-head DMAs are contiguous
    wq8 = nc.dram_tensor("wq8", [HPC, 128, HC, 128], F8, kind="ExternalInput").ap()
    wk8 = nc.dram_tensor("wk8", [HPC, 128, HC, 128], F8, kind="ExternalInput").ap()
    wv = nc.dram_tensor("wv", [128, HC, FQ], BF16, kind="ExternalInput").ap()
    wo = nc.dram_tensor("wo", [128, HC, FQ], BF16, kind="ExternalInput").ap()
    cosT = nc.dram_tensor("cosT", [128, TOK], BF16, kind="ExternalInput").ap()
    sinT = nc.dram_tensor("sinT", [128, TOK], BF16, kind="ExternalInput").ap()
    masks = nc.dram_tensor("masks", [128, 4, QB], BF16, kind="ExternalInput").ap()
    ones_col = nc.dram_tensor("ones_col", [128, 1], BF16, kind="ExternalInput").ap()
    ones_row = nc.dram_tensor("ones_row", [1, 128], F32R, kind="ExternalInput").ap()
    out = nc.dram_tensor("out", [TOK, FQ], BF16, kind="ExternalOutput").ap()

    with tile.TileContext(nc) as tc, nc.allow_low_precision(reason="fp8/bf16 kernel"):
        with tc.tile_pool(name="dram", bufs=1, space="DRAM") as dram, \
             tc.tile_pool(name="dsh", bufs=1, space="DRAM") as dsh, \
             tc.tile_pool(name="wconst", bufs=1) as wconst:
            aloc = [[dram.tile([FQ, 2 * QB], BF16, name=f"aloc{b_}{h_}")
                     for h_ in range(2)] for b_ in range(B)]
            agth = [[dsh.tile([H, 2 * QB], BF16, addr_space="Shared",
                              name=f"agth{b_}{h_}")
                     for h_ in range(2)] for b_ in range(B)]

            # resident weights + small constants
            wq_sb = wconst.tile([128, HPC, HC, 128], F8)
            wk_sb = wconst.tile([128, HPC, HC, 128], F8)
            wv_sb = wconst.tile([128, HC, FQ], BF16)
            wo_sb = wconst.tile([128, HC, FQ], BF16)
            mask_sb = wconst.tile([128, 4, QB], BF16)
            ones_sb = wconst.tile([128, 1], BF16)
            onesr_sb = wconst.tile([1, 128], F32R)

            with tc.tile_pool(name="xq", bufs=2) as xqp, \
                 tc.tile_pool(name="xb", bufs=3) as xbp, \
                 tc.tile_pool(name="tbl", bufs=2) as tblp, \
                 tc.tile_pool(name="qkv", bufs=1) as qkvp, \
                 tc.tile_pool(name="rp", bufs=1) as rpp, \
                 tc.tile_pool(name="ev", bufs=3) as evp, \
                 tc.tile_pool(name="att", bufs=2) as attp, \
                 tc.tile_pool(name="psP", bufs=2, space="PSUM") as psP, \
                 tc.tile_pool(name="psS", bufs=2, space="PSUM") as psS, \
                 tc.tile_pool(name="psX", bufs=1, space="PSUM") as psX:

                # per-batch SBUF q/k/v (reused between batches)
                q_sb = qkvp.tile([128, HPC, S], F8, name="q_sb")
                k_sb = qkvp.tile([128, HPC, S], F8, name="k_sb")
                v_sb = qkvp.tile([128, S // 128, FQ], BF16, name="v_sb")

                def load_qk_weights():
                    # per-head pieces; head 0 first so chunk 0 can start early
                    for hl in range(HPC):
                        nc.sync.dma_start(wq_sb[:, hl, :, :], wq8[hl])
                    for hl in range(HPC):
                        nc.sync.dma_start(wk_sb[:, hl, :, :], wk8[hl])

                def qkv_batch(b_i):
                    for g in range(NTC // 2):       # 1024-token groups
                        xqs = []
                        for ci in range(2):
                            ch = 2 * g + ci
                            cidx = b_i * NTC + ch
                            xq = xqp.tile([128, HC, TCW], F8, tag="xq", name="xq")
                            nc.sync.dma_start(xq[:], xq8[:, cidx, :, :])
                            xqs.append(xq)
                        t0 = b_i * S + g * 2 * TCW
                        cs = tblp.tile([128, 2, TCW], BF16, tag="cs", name="cs")
                        sn = tblp.tile([128, 2, TCW], BF16, tag="sn", name="sn")
                        nc.sync.dma_start(
                            cs.rearrange("p a b -> p (a b)"), cosT[:, t0:t0 + 2 * TCW])
                        nc.sync.dma_start(
                            sn.rearrange("p a b -> p (a b)"), sinT[:, t0:t0 + 2 * TCW])

                        # Q/K: fp8 DoubleRow, two 512-token chunks per weight
                        for f in range(8):
                            w_sb = wq_sb if f < 4 else wk_sb
                            hl = f % 4
                            pq = [psP.tile([128, TCW], F32, tag="po", name="pq")
                                  for _ in range(2)]
                            for h2 in range(HC // 2):
                                for ci in range(2):
                                    nc.tensor.matmul(
                                        pq[ci][:],
                                        w_sb[:, hl, 2 * h2:2 * h2 + 2, :],
                                        xqs[ci][:, 2 * h2:2 * h2 + 2, :],
                                        start=(h2 == 0), stop=(h2 == HC // 2 - 1),
                                        perf_mode=DR)
                            dst = (q_sb if f < 4 else k_sb)
                            for ci in range(2):
                                tmp = rpp.tile([128, TCW], F32, tag="tmp", name="tmp")
                                sw = rpp.tile([128, TCW], F32, tag="sw", name="sw")
                                nc.vector.tensor_mul(tmp[:], pq[ci][:], cs[:, ci, :])
                                nc.vector.tensor_mul(
                                    sw[0:64, :], pq[ci][64:128, :], sn[0:64, ci, :])
                                nc.vector.tensor_mul(
                                    sw[64:128, :], pq[ci][0:64, :], sn[64:128, ci, :])
                                tt = (2 * g + ci) * TCW
                                nc.vector.tensor_add(
                                    dst[:, hl, tt:tt + TCW], tmp[:], sw[:])

                        # V: bf16, out [tok=128, f=512] per token tile
                        for ci in range(2):
                            ch = 2 * g + ci
                            cidx = b_i * NTC + ch
                            pv = [psS.tile([128, 2, FQ], F32, tag="sc", name=f"pv{i}")
                                  for i in range(2)]
                            for hc in range(HC):
                                xb = xbp.tile([128, TCW], BF16, tag="xb", name="xb")
                                nc.sync.dma_start(xb[:], xbf[:, cidx, hc, :])
                                for ts in range(4):
                                    nc.tensor.matmul(
                                        pv[ts // 2][:, ts % 2, :],
                                        xb[:, ts * 128:(ts + 1) * 128],
                                        wv_sb[:, hc, :],
                                        start=(hc == 0), stop=(hc == HC - 1))
                            for ts in range(4):
                                nc.scalar.copy(
                                    v_sb[:, ch * 4 + ts, :], pv[ts // 2][:, ts % 2, :])

                def attn_batch(b_i):
                    # task list: (j, hl, p) kv-pair tasks in block order
                    plist = []
                    for j in range(S // QB):
                        for hl in range(HPC):
                            for p in range(2 * (j + 1)):
                                plist.append((j, hl, p))

                    state = {}   # per-block live psum tiles

                    def emit_scores(task):
                        j, hl, p = task
                        sc = psS.tile([128, 2, QB], F32, tag="sc", name="sc")
                        q_rhs = q_sb[:, hl, j * QB:(j + 1) * QB]
                        for ci in range(2):
                            c = 2 * p + ci
                            nc.tensor.matmul(
                                sc[:, ci, :],
                                k_sb[:, hl, c * 128:(c + 1) * 128],
                                q_rhs, start=True, stop=True)
                        return sc

                    def emit_consume(sc, task):
                        j, hl, p = task
                        npair = 2 * (j + 1)
                        pt = attp.tile([128, 2, QB], BF16, tag="pt", name="pt")
                        dr0 = 2 * p - 4 * j
                        if dr0 >= 0:
                            et = attp.tile([128, 2, QB], BF16, tag="pt", name="et")
                            nc.scalar.activation(
                                et[:], sc[:], mybir.ActivationFunctionType.Exp,
                                scale=SEXP)
                            nc.vector.tensor_mul(
                                pt[:], et[:], mask_sb[:, dr0:dr0 + 2, :])
                        else:
                            nc.scalar.activation(
                                pt[:], sc[:], mybir.ActivationFunctionType.Exp,
                                scale=SEXP)
                        if p == 0:
                            state[(j, hl)] = (
                                psP.tile([128, QB], F32, tag="po", name="po"),
                                psX.tile([1, QB], F32, tag="ps", name="ps"))
                        po, ps = state[(j, hl)]
                        for ci in range(2):
                            first = (p == 0 and ci == 0)
                            last = (p == npair - 1 and ci == 1)
                            c = 2 * p + ci
                            nc.tensor.matmul(
                                po[:], v_sb[:, c, hl * 128:(hl + 1) * 128],
                                pt[:, ci, :], start=first, stop=last)
                            nc.tensor.matmul(
                                ps[:], ones_sb[:], pt[:, ci, :],
                                start=first, stop=last)

                    def emit_finalize(blk):
                        j, hl = blk
                        po, ps = state.pop(blk)
                        r_sb = attp.tile([1, QB], F32R, tag="r", name="r_sb")
                        nc.vector.reciprocal(r_sb[:], ps[:])
                        pb = psX.tile([128, QB], F32, tag="pb", name="pb")
                        nc.tensor.matmul(
                            pb[:], onesr_sb[:], r_sb[:], start=True, stop=True)
                        bsb = attp.tile([128, QB], F32, tag="bsb", name="bsb")
                        nc.vector.tensor_copy(bsb[:], pb[:])
                        o_sb = attp.tile([128, QB], BF16, tag="osb", name="o_sb")
                        nc.vector.tensor_mul(o_sb[:], po[:], bsb[:])
                        nc.sync.dma_start(
                            aloc[b_i][j // 2][hl * 128:(hl + 1) * 128,
                                              (j % 2) * QB:(j % 2 + 1) * QB],
                            o_sb[:])
                        if hl == HPC - 1 and j % 2 == 1:
                            nc.gpsimd.collective_compute(
                                "AllGather",
                                mybir.AluOpType.bypass,
                                ins=[aloc[b_i][j // 2].opt()],
                                outs=[agth[b_i][j // 2].opt()],
                                replica_groups=[list(range(CORES))],
                            )

                    sc_next = emit_scores(plist[0])
                    pending_fin = None
                    for i, task in enumerate(plist):
                        sc_cur = sc_next
                        if i + 1 < len(plist):
                            sc_next = emit_scores(plist[i + 1])
                        if pending_fin is not None:
                            emit_finalize(pending_fin)
                            pending_fin = None
                        emit_consume(sc_cur, task)
                        j, hl, p = task
                        if p == 2 * (j + 1) - 1:
                            pending_fin = (j, hl)
                    emit_finalize(pending_fin)

                def oproj_batch(b_i):
                    for h2 in range(2):
                        for tt in range(2):
                            pf = [psS.tile([128, 2, FQ], F32, tag="sc", name=f"pf{i}")
                                  for i in range(2)]
                            for k in range(HC):
                                agr = evp.tile([128, QB], BF16, tag="agr", name="agr")
                                nc.sync.dma_start(
                                    agr[:],
                                    agth[b_i][h2][k * 128:(k + 1) * 128,
                                                  tt * QB:(tt + 1) * QB])
                                for ts in range(4):
                                    nc.tensor.matmul(
                                        pf[ts // 2][:, ts % 2, :],
                                        agr[:, ts * 128:(ts + 1) * 128],
                                        wo_sb[:, k, :],
                                        start=(k == 0), stop=(k == HC - 1))
                            t0 = b_i * S + h2 * 2 * QB + tt * QB
                            for ts in range(4):
                                fo = evp.tile([128, FQ], BF16, tag="fo", name="fo")
                                if ts < 2:
                                    nc.scalar.copy(fo[:], pf[ts // 2][:, ts % 2, :])
                                else:
                                    nc.vector.tensor_copy(fo[:], pf[ts // 2][:, ts % 2, :])
                                nc.sync.dma_start(
                                    out[t0 + ts * 128:t0 + (ts + 1) * 128, :], fo[:])

                # weight/const DMAs, ordered by first use
                load_qk_weights()
                nc.sync.dma_start(wv_sb[:], wv)
                nc.sync.dma_start(mask_sb[:], masks)
                nc.sync.dma_start(ones_sb[:], ones_col)
                nc.sync.dma_start(onesr_sb[:], ones_row)

                with nc.named_scope("qkv_a"):
                    qkv_batch(0)
                with nc.named_scope("attn0"):
                    attn_batch(0)
                with nc.named_scope("qkv_b"):
                    qkv_batch(1)
                nc.sync.dma_start(wo_sb[:], wo)
                with nc.named_scope("attn1"):
                    attn_batch(1)
                with nc.named_scope("oproj0"):
                    oproj_batch(0)
                with nc.named_scope("oproj1"):
                    oproj_batch(1)

    nc.compile()
    return nc


def _get_nc():
    if "nc" not in _CACHE:
        _CACHE["nc"] = _build()
    return _CACHE["nc"]


def _chunked(a):
    """[H, N] -> [128, HC, N] with dim1 = feature chunk."""
    return np.ascontiguousarray(
        a.reshape(HC, 128, a.shape[1]).transpose(1, 0, 2))


def _chunked_x(a):
    """[H, TOK] -> [128, B*NTC, HC, TCW] (token-chunked, feature-chunked)."""
    return np.ascontiguousarray(
        a.reshape(HC, 128, B * NTC, TCW).transpose(1, 2, 0, 3))


def _headmajor(a):
    """[H, FQ] -> [HPC, 128, HC, 128]: per-head contiguous weight blocks."""
    # a[h, f]; h = hc*128 + p; f = hl*128 + c
    return np.ascontiguousarray(
        a.reshape(HC, 128, HPC, 128).transpose(2, 1, 0, 3))


def kernel(positions, hidden_states, w_pack, w_o):
    global LAST_RESULTS
    nc = _get_nc()

    x = np.asarray(hidden_states, dtype=np.float32).reshape(TOK, H)
    w_pack = np.asarray(w_pack, dtype=np.float32)
    w_o = np.asarray(w_o, dtype=np.float32)
    pos_flat = np.asarray(positions).reshape(-1).astype(np.float64)  # [TOK]

    xT = x.T  # [H, TOK]
    xq8_full = _chunked_x((xT * XS).astype(ml_dtypes.float8_e4m3))
    xbf_full = _chunked_x(xT.astype(ml_dtypes.bfloat16))

    half = D // 2
    inv = 1.0 / (ROPE_THETA ** (np.arange(half, dtype=np.float64) * 2.0 / D))
    f = np.outer(inv, pos_flat)                        # [64, TOK]
    cos = np.cos(f)
    sin = np.sin(f)
    tscale = QS / (XS * WS)
    cosT = (np.concatenate([cos, cos], axis=0) * tscale).astype(ml_dtypes.bfloat16)
    sinT = (np.concatenate([-sin, sin], axis=0) * tscale).astype(ml_dtypes.bfloat16)

    kvi = np.arange(128)[:, None, None]
    rr = np.arange(4)[None, :, None]
    qi = np.arange(QB)[None, None, :]
    masks = ((kvi + 128 * rr) <= qi).astype(ml_dtypes.bfloat16)

    ones_col = np.ones((128, 1), ml_dtypes.bfloat16)
    ones_row = np.ones((1, 128), np.float32)

    in_maps = []
    for c in range(CORES):
        wq = (w_pack[:, FQ * c:FQ * (c + 1)] * WS).astype(ml_dtypes.float8_e4m3)
        wk = (w_pack[:, H + FQ * c:H + FQ * (c + 1)] * WS).astype(ml_dtypes.float8_e4m3)
        wvc = w_pack[:, 2 * H + FQ * c:2 * H + FQ * (c + 1)].astype(ml_dtypes.bfloat16)
        woc = w_o[:, FQ * c:FQ * (c + 1)].astype(ml_dtypes.bfloat16)
        in_maps.append({
            "xq8": xq8_full,
            "xbf": xbf_full,
            "wq8": _headmajor(wq),
            "wk8": _headmajor(wk),
            "wv": _chunked(wvc),
            "wo": _chunked(woc),
            "cosT": cosT, "sinT": sinT,
            "masks": masks, "ones_col": ones_col, "ones_row": ones_row,
        })

    res = bass_utils.run_bass_kernel_spmd(nc, in_maps, core_ids=list(range(CORES)))
    LAST_RESULTS = res
    outs = [np.asarray(res.results[c]["out"]).astype(np.float32)
            for c in range(CORES)]
    return np.concatenate(outs, axis=1).reshape(B, S, H)


# revision 15
# speedup vs baseline: 1.6790x; 1.0620x over previous
"""BaiChuan attention block on 8 Trainium2 NeuronCores.

Sharding: tensor-parallel over heads (4 heads/core). Each core computes its
512-wide q/k/v slices for all 4096 tokens, runs attention for its 4 heads on
both batches, AllGathers attention outputs (feature-major, bf16) per
half-batch, and computes a 512-wide output-feature slice of o_proj for all
tokens; the host concatenates slices.

Layout strategy: x is pre-transposed on the host to feature-major (xT), so no
PE transposes are needed anywhere. Weights live in SBUF for the whole kernel
(loaded once). q/k/v stay in SBUF per batch (no DRAM roundtrip).

Precision: Q/K projection runs in fp8 (e4m3) with DoubleRow packing; the
softmax only depends on score differences, which are tiny for this data, so
fp8 rounding of q/k is far below the output tolerance. Scales: x*XS and w*WS
are folded out through the RoPE tables; q/k are stored in SBUF as fp8 scaled
by QS, and D**-0.5 / QS**2 is applied via the exp's scale argument. The V
path, attention values, and o_proj run in bf16 with fp32 PSUM accumulation.

Attention is software-pipelined: score matmuls run one kv-pair ahead of the
exp/PV/sum consumers, and each block's normalization (reciprocal + broadcast)
is deferred one pair-slot so the PE never waits on the DVE chain.
"""
import numpy as np
import ml_dtypes

import concourse.bass as bass
import concourse.mybir as mybir
import concourse.tile as tile
from concourse import bacc, bass_utils

# Problem dims (hardcoded per contest contract)
B, S, H, NH = 2, 2048, 4096, 32
D = H // NH            # 128 head dim
CORES = 8
HPC = NH // CORES      # 4 heads per core
TOK = B * S            # 4096 tokens
FQ = HPC * D           # 512 per-core q/k/v feature width
TCW = 512              # token chunk width for QKV phase
NTC = S // TCW         # 4 chunks per batch
HC = H // 128          # 32 contraction chunks
QB = 512               # attention q block
ROPE_THETA = 10000.0

# fp8 scale plan
XS = 32.0              # x pre-scale before fp8 quantization
WS = 32.0              # wq/wk pre-scale before fp8 quantization
QS = 16.0              # q/k SBUF storage scale
SEXP = float(D ** -0.5 / (QS * QS))  # exp() input scale

F32 = mybir.dt.float32
F32R = mybir.dt.float32r
BF16 = mybir.dt.bfloat16
F8 = mybir.dt.float8e4
DR = mybir.MatmulPerfMode.DoubleRow

_CACHE = {}
LAST_RESULTS = None


def _build():
    nc = bacc.Bacc("TRN2", target_bir_lowering=False, debug=False, num_devices=CORES)

    # x: [128, NCHUNK, HC, TCW], pre-chunked so per-chunk DMAs are contiguous
    xq8 = nc.dram_tensor("xq8", [128, B * NTC, HC, TCW], F8, kind="ExternalInput").ap()
    xbf = nc.dram_tensor("xbf", [128, B * NTC, HC, TCW], BF16, kind="ExternalInput").ap()
    # wq/wk: head-major [4, 128, HC, 128] so per-head DMAs are contiguous
    wq8 = nc.dram_tensor("wq8", [HPC, 128, HC, 128], F8, kind="ExternalInput").ap()
    wk8 = nc.dram_tensor("wk8", [HPC, 128, HC, 128], F8, kind="ExternalInput").ap()
    wv = nc.dram_tensor("wv", [128, HC, FQ], BF16, kind="ExternalInput").ap()
    wo = nc.dram_tensor("wo", [128, HC, FQ], BF16, kind="ExternalInput").ap()
    cosT = nc.dram_tensor("cosT", [128, TOK], BF16, kind="ExternalInput").ap()
    sinT = nc.dram_tensor("sinT", [128, TOK], BF16, kind="ExternalInput").ap()
    masks = nc.dram_tensor("masks", [128, 4, QB], BF16, kind="ExternalInput").ap()
    ones_col = nc.dram_tensor("ones_col", [128, 1], BF16, kind="ExternalInput").ap()
    ones_row = nc.dram_tensor("ones_row", [1, 128], BF16, kind="ExternalInput").ap()
    out = nc.dram_tensor("out", [TOK, FQ], BF16, kind="ExternalOutput").ap()

    with tile.TileContext(nc) as tc, nc.allow_low_precision(reason="fp8/bf16 kernel"):
        with tc.tile_pool(name="dram", bufs=1, space="DRAM") as dram, \
             tc.tile_pool(name="dsh", bufs=1, space="DRAM") as dsh, \
             tc.tile_pool(name="wconst", bufs=1) as wconst:
            aloc = [[dram.tile([FQ, 2 * QB], BF16, name=f"aloc{b_}{h_}")
                     for h_ in range(2)] for b_ in range(B)]
            agth = [[dsh.tile([H, 2 * QB], BF16, addr_space="Shared",
                              name=f"agth{b_}{h_}")
                     for h_ in range(2)] for b_ in range(B)]

            # resident weights + small constants
            wq_sb = wconst.tile([128, HPC, HC, 128], F8)
            wk_sb = wconst.tile([128, HPC, HC, 128], F8)
            wv_sb = wconst.tile([128, HC, FQ], BF16)
            wo_sb = wconst.tile([128, HC, FQ], BF16)
            mask_sb = wconst.tile([128, 4, QB], BF16)
            ones_sb = wconst.tile([128, 1], BF16)
            onesr_sb = wconst.tile([1, 128], BF16)

            with tc.tile_pool(name="xq", bufs=2) as xqp, \
                 tc.tile_pool(name="xb", bufs=3) as xbp, \
                 tc.tile_pool(name="tbl", bufs=2) as tblp, \
                 tc.tile_pool(name="qkv", bufs=1) as qkvp, \
                 tc.tile_pool(name="rp", bufs=1) as rpp, \
                 tc.tile_pool(name="ev", bufs=3) as evp, \
                 tc.tile_pool(name="att", bufs=2) as attp, \
                 tc.tile_pool(name="psP", bufs=2, space="PSUM") as psP, \
                 tc.tile_pool(name="psS", bufs=4, space="PSUM") as psS, \
                 tc.tile_pool(name="psX", bufs=1, space="PSUM") as psX:

                # per-batch SBUF q/k/v (reused between batches)
                q_sb = qkvp.tile([128, HPC, S], F8, name="q_sb")
                k_sb = qkvp.tile([128, HPC, S], F8, name="k_sb")
                v_sb = qkvp.tile([128, S // 128, FQ], BF16, name="v_sb")

                def load_qk_weights():
                    # per-head pieces; head 0 first so chunk 0 can start early
                    for hl in range(HPC):
                        nc.sync.dma_start(wq_sb[:, hl, :, :], wq8[hl])
                    for hl in range(HPC):
                        nc.sync.dma_start(wk_sb[:, hl, :, :], wk8[hl])

                def qkv_batch(b_i):
                    for g in range(NTC // 2):       # 1024-token groups
                        xqs = []
                        for ci in range(2):
                            ch = 2 * g + ci
                            cidx = b_i * NTC + ch
                            xq = xqp.tile([128, HC, TCW], F8, tag="xq", name="xq")
                            nc.sync.dma_start(xq[:], xq8[:, cidx, :, :])
                            xqs.append(xq)
                        t0 = b_i * S + g * 2 * TCW
                        cs = tblp.tile([128, 2, TCW], BF16, tag="cs", name="cs")
                        sn = tblp.tile([128, 2, TCW], BF16, tag="sn", name="sn")
                        nc.sync.dma_start(
                            cs.rearrange("p a b -> p (a b)"), cosT[:, t0:t0 + 2 * TCW])
                        nc.sync.dma_start(
                            sn.rearrange("p a b -> p (a b)"), sinT[:, t0:t0 + 2 * TCW])

                        # Q/K: fp8 DoubleRow, two 512-token chunks per weight
                        for f in range(8):
                            w_sb = wq_sb if f < 4 else wk_sb
                            hl = f % 4
                            pq = [psP.tile([128, TCW], F32, tag="po", name="pq")
                                  for _ in range(2)]
                            for h2 in range(HC // 2):
                                for ci in range(2):
                                    nc.tensor.matmul(
                                        pq[ci][:],
                                        w_sb[:, hl, 2 * h2:2 * h2 + 2, :],
                                        xqs[ci][:, 2 * h2:2 * h2 + 2, :],
                                        start=(h2 == 0), stop=(h2 == HC // 2 - 1),
                                        perf_mode=DR)
                            dst = (q_sb if f < 4 else k_sb)
                            for ci in range(2):
                                tmp = rpp.tile([128, TCW], F32, tag="tmp", name="tmp")
                                sw = rpp.tile([128, TCW], F32, tag="sw", name="sw")
                                nc.vector.tensor_mul(tmp[:], pq[ci][:], cs[:, ci, :])
                                nc.vector.tensor_mul(
                                    sw[0:64, :], pq[ci][64:128, :], sn[0:64, ci, :])
                                nc.vector.tensor_mul(
                                    sw[64:128, :], pq[ci][0:64, :], sn[64:128, ci, :])
                                tt = (2 * g + ci) * TCW
                                nc.vector.tensor_add(
                                    dst[:, hl, tt:tt + TCW], tmp[:], sw[:])

                        # V: bf16, out [tok=128, f=512] per token tile
                        for ci in range(2):
                            ch = 2 * g + ci
                            cidx = b_i * NTC + ch
                            pv = [psS.tile([128, FQ], F32, tag="sc", name=f"pv{i}")
                                  for i in range(4)]
                            for hc in range(HC):
                                xb = xbp.tile([128, TCW], BF16, tag="xb", name="xb")
                                nc.sync.dma_start(xb[:], xbf[:, cidx, hc, :])
                                for ts in range(4):
                                    nc.tensor.matmul(
                                        pv[ts][:],
                                        xb[:, ts * 128:(ts + 1) * 128],
                                        wv_sb[:, hc, :],
                                        start=(hc == 0), stop=(hc == HC - 1))
                            for ts in range(4):
                                nc.scalar.copy(
                                    v_sb[:, ch * 4 + ts, :], pv[ts][:])

                def attn_batch(b_i):
                    # task list: (j, hl, c) kv-chunk tasks in block order
                    plist = []
                    for j in range(S // QB):
                        for hl in range(HPC):
                            for c in range(4 * (j + 1)):
                                plist.append((j, hl, c))

                    state = {}   # per-block live psum tiles

                    def emit_scores(task):
                        j, hl, c = task
                        sc = psS.tile([128, QB], F32, tag="sc", name="sc")
                        nc.tensor.matmul(
                            sc[:],
                            k_sb[:, hl, c * 128:(c + 1) * 128],
                            q_sb[:, hl, j * QB:(j + 1) * QB],
                            start=True, stop=True)
                        return sc

                    def emit_consume(sc, task):
                        j, hl, c = task
                        nchunk = 4 * (j + 1)
                        pt = attp.tile([128, QB], BF16, tag="pt", name="pt")
                        dr = c - 4 * j
                        if dr >= 0:
                            et = attp.tile([128, QB], BF16, tag="pt", name="et")
                            nc.scalar.activation(
                                et[:], sc[:], mybir.ActivationFunctionType.Exp,
                                scale=SEXP)
                            nc.vector.tensor_mul(
                                pt[:], et[:], mask_sb[:, dr, :])
                        else:
                            nc.scalar.activation(
                                pt[:], sc[:], mybir.ActivationFunctionType.Exp,
                                scale=SEXP)
                        if c == 0:
                            state[(j, hl)] = (
                                psP.tile([128, QB], F32, tag="po", name="po"),
                                psX.tile([1, QB], F32, tag="ps", name="ps"))
                        po, ps = state[(j, hl)]
                        first = (c == 0)
                        last = (c == nchunk - 1)
                        nc.tensor.matmul(
                            po[:], v_sb[:, c, hl * 128:(hl + 1) * 128],
                            pt[:], start=first, stop=last)
                        nc.tensor.matmul(
                            ps[:], ones_sb[:], pt[:], start=first, stop=last)

                    def emit_finalize(blk):
                        j, hl = blk
                        po, ps = state.pop(blk)
                        ps_sb = attp.tile([1, QB], BF16, tag="r", name="ps_sb")
                        nc.scalar.copy(ps_sb[:], ps[:])
                        pb = psX.tile([128, QB], F32, tag="pb", name="pb")
                        nc.tensor.matmul(
                            pb[:], onesr_sb[:], ps_sb[:],
                            start=True, stop=True)
                        rb = attp.tile([128, QB], F32, tag="rb", name="rb")
                        nc.vector.reciprocal_approx_fast(rb[:], pb[:])
                        o_sb = attp.tile([128, QB], BF16, tag="osb", name="o_sb")
                        nc.vector.tensor_mul(o_sb[:], po[:], rb[:])
                        nc.sync.dma_start(
                            aloc[b_i][j // 2][hl * 128:(hl + 1) * 128,
                                              (j % 2) * QB:(j % 2 + 1) * QB],
                            o_sb[:])
                        if hl == HPC - 1 and j % 2 == 1:
                            nc.gpsimd.collective_compute(
                                "AllGather",
                                mybir.AluOpType.bypass,
                                ins=[aloc[b_i][j // 2].opt()],
                                outs=[agth[b_i][j // 2].opt()],
                                replica_groups=[list(range(CORES))],
                            )

                    DEPTH = 2
                    scq = [emit_scores(plist[i]) for i in range(DEPTH)]
                    pending_fin = None
                    for i, task in enumerate(plist):
                        if i + DEPTH < len(plist):
                            scq.append(emit_scores(plist[i + DEPTH]))
                        if pending_fin is not None:
                            emit_finalize(pending_fin)
                            pending_fin = None
                        emit_consume(scq.pop(0), task)
                        j, hl, c = task
                        if c == 4 * (j + 1) - 1:
                            pending_fin = (j, hl)
                    emit_finalize(pending_fin)

                def oproj_batch(b_i):
                    for h2 in range(2):
                        for tt in range(2):
                            pf = [psS.tile([128, FQ], F32, tag="sc", name=f"pf{i}")
                                  for i in range(4)]
                            for k in range(HC):
                                agr = evp.tile([128, QB], BF16, tag="agr", name="agr")
                                nc.sync.dma_start(
                                    agr[:],
                                    agth[b_i][h2][k * 128:(k + 1) * 128,
                                                  tt * QB:(tt + 1) * QB])
                                for ts in range(4):
                                    nc.tensor.matmul(
                                        pf[ts][:],
                                        agr[:, ts * 128:(ts + 1) * 128],
                                        wo_sb[:, k, :],
                                        start=(k == 0), stop=(k == HC - 1))
                            t0 = b_i * S + h2 * 2 * QB + tt * QB
                            for ts in range(4):
                                fo = evp.tile([128, FQ], BF16, tag="fo", name="fo")
                                if ts < 2:
                                    nc.scalar.copy(fo[:], pf[ts][:])
                                else:
                                    nc.vector.tensor_copy(fo[:], pf[ts][:])
                                nc.sync.dma_start(
                                    out[t0 + ts * 128:t0 + (ts + 1) * 128, :], fo[:])

                # weight/const DMAs, ordered by first use
                load_qk_weights()
                nc.sync.dma_start(wv_sb[:], wv)
                nc.sync.dma_start(mask_sb[:], masks)
                nc.sync.dma_start(ones_sb[:], ones_col)
                nc.sync.dma_start(onesr_sb[:], ones_row)

                with nc.named_scope("qkv_a"):
                    qkv_batch(0)
                with nc.named_scope("attn0"):
                    attn_batch(0)
                with nc.named_scope("qkv_b"):
                    qkv_batch(1)
                nc.sync.dma_start(wo_sb[:], wo)
                with nc.named_scope("attn1"):
                    attn_batch(1)
                with nc.named_scope("oproj0"):
                    oproj_batch(0)
                with nc.named_scope("oproj1"):
                    oproj_batch(1)

    nc.compile()
    return nc


def _get_nc():
    if "nc" not in _CACHE:
        _CACHE["nc"] = _build()
    return _CACHE["nc"]


def _chunked(a):
    """[H, N] -> [128, HC, N] with dim1 = feature chunk."""
    return np.ascontiguousarray(
        a.reshape(HC, 128, a.shape[1]).transpose(1, 0, 2))


def _chunked_x(a):
    """[H, TOK] -> [128, B*NTC, HC, TCW] (token-chunked, feature-chunked)."""
    return np.ascontiguousarray(
        a.reshape(HC, 128, B * NTC, TCW).transpose(1, 2, 0, 3))


def _headmajor(a):
    """[H, FQ] -> [HPC, 128, HC, 128]: per-head contiguous weight blocks."""
    # a[h, f]; h = hc*128 + p; f = hl*128 + c
    return np.ascontiguousarray(
        a.reshape(HC, 128, HPC, 128).transpose(2, 1, 0, 3))


def kernel(positions, hidden_states, w_pack, w_o):
    global LAST_RESULTS
    nc = _get_nc()

    x = np.asarray(hidden_states, dtype=np.float32).reshape(TOK, H)
    w_pack = np.asarray(w_pack, dtype=np.float32)
    w_o = np.asarray(w_o, dtype=np.float32)
    pos_flat = np.asarray(positions).reshape(-1).astype(np.float64)  # [TOK]

    xT = x.T  # [H, TOK]
    xq8_full = _chunked_x((xT * XS).astype(ml_dtypes.float8_e4m3))
    xbf_full = _chunked_x(xT.astype(ml_dtypes.bfloat16))

    half = D // 2
    inv = 1.0 / (ROPE_THETA ** (np.arange(half, dtype=np.float64) * 2.0 / D))
    f = np.outer(inv, pos_flat)                        # [64, TOK]
    cos = np.cos(f)
    sin = np.sin(f)
    tscale = QS / (XS * WS)
    cosT = (np.concatenate([cos, cos], axis=0) * tscale).astype(ml_dtypes.bfloat16)
    sinT = (np.concatenate([-sin, sin], axis=0) * tscale).astype(ml_dtypes.bfloat16)

    kvi = np.arange(128)[:, None, None]
    rr = np.arange(4)[None, :, None]
    qi = np.arange(QB)[None, None, :]
    masks = ((kvi + 128 * rr) <= qi).astype(ml_dtypes.bfloat16)

    ones_col = np.ones((128, 1), ml_dtypes.bfloat16)
    ones_row = np.ones((1, 128), ml_dtypes.bfloat16)

    in_maps = []
    for c in range(CORES):
        wq = (w_pack[:, FQ * c:FQ * (c + 1)] * WS).astype(ml_dtypes.float8_e4m3)
        wk = (w_pack[:, H + FQ * c:H + FQ * (c + 1)] * WS).astype(ml_dtypes.float8_e4m3)
        wvc = w_pack[:, 2 * H + FQ * c:2 * H + FQ * (c + 1)].astype(ml_dtypes.bfloat16)
        woc = w_o[:, FQ * c:FQ * (c + 1)].astype(ml_dtypes.bfloat16)
        in_maps.append({
            "xq8": xq8_full,
            "xbf": xbf_full,
            "wq8": _headmajor(wq),
            "wk8": _headmajor(wk),
            "wv": _chunked(wvc),
            "wo": _chunked(woc),
            "cosT": cosT, "sinT": sinT,
            "masks": masks, "ones_col": ones_col, "ones_row": ones_row,
        })

    res = bass_utils.run_bass_kernel_spmd(nc, in_maps, core_ids=list(range(CORES)))
    LAST_RESULTS = res
    outs = [np.asarray(res.results[c]["out"]).astype(np.float32)
            for c in range(CORES)]
    return np.concatenate(outs, axis=1).reshape(B, S, H)
